# revision 29
# baseline (speedup 1.0000x reference)
"""Distributed multi-head attention kernel for 8 TRN2 NeuronCores, v2.

Problem: x(4,2048,1024) -> qkv proj (w_qkv 3072x1024) -> 16-head attention
(head_dim 64, softmax) -> out proj (w_out 1024x1024 + b_out).

Sharding: head-parallel. Core c owns heads {2c, 2c+1}: it computes Q/K/V for
those heads over all 8192 tokens, runs attention, then a per-batch AllToAll
(1MB bf16) converts the head-sharded attention output into a token-sharded
layout (256 tokens/core/batch, all 16 heads) for the output projection.

v2 restructure vs v1: the ScalarE exp stream (~264us of activations) is the
pacing resource; v1 idled it ~7.5us at every QKV token-tile and ~18us at batch
boundaries because QKV(b+1)/outproj(b-1) ran as serial phases. v2 emits them
as "filler" units (QK half-tile, V half-tile, outproj m-tile) interleaved into
attention(b)'s exp-paced group stream every 3 groups, sharing the 3-bank
"big3" PSUM ring with the S tiles. q-tile finishing carries across batch
boundaries so the exp stream never drains.
"""

import numpy as np
import ml_dtypes

import concourse.bass as bass
import concourse.mybir as mybir
import concourse.tile as tile
from concourse import bacc, bass_utils
from concourse.tile import add_dep_helper

FP32 = mybir.dt.float32
BF16 = mybir.dt.bfloat16
AF = mybir.ActivationFunctionType

N_CORES = 8
B, NTOK, D = 4, 2048, 1024
T = B * NTOK  # 8192 tokens total
NH, HD = 16, 64
HL = NH // N_CORES  # 2 heads per core
SCALE = float(HD) ** -0.5  # 0.125
TN = 512  # token tile for QKV / q tile for attention
NT = T // TN  # 16
KC = D // 128  # 8 contraction chunks for projections
KT = NTOK // 128  # 16 k-chunks per batch in attention
TPB = NTOK // N_CORES  # 256 tokens per (core, batch) after A2A
TPC = T // N_CORES  # 1024 tokens per core total
WCOLS = 3 * HL * HD  # 384 qkv output dims per core


def build_nc(debug=False):
    nc = bacc.Bacc(
        "TRN2", target_bir_lowering=False, debug=False, num_devices=N_CORES
    )
    xt = nc.dram_tensor("xt", [D, T], BF16, kind="ExternalInput").ap()
    wt = nc.dram_tensor("wt", [D, WCOLS], BF16, kind="ExternalInput").ap()
    wo = nc.dram_tensor("wo", [D, D], BF16, kind="ExternalInput").ap()
    bias = nc.dram_tensor("bias", [1, D], FP32, kind="ExternalInput").ap()
    # row r of out = batch r//TPB, token (core * TPB + r % TPB) of that batch
    out = nc.dram_tensor("out", [TPC, D], FP32, kind="ExternalOutput").ap()

    with tile.TileContext(nc) as tc:
        with (
            tc.tile_pool(name="const", bufs=1) as const,
            tc.tile_pool(name="xin", bufs=24) as xin,
            tc.tile_pool(name="probs", bufs=6) as probs,
            tc.tile_pool(name="norm", bufs=6) as norm,
            tc.tile_pool(name="ot", bufs=6) as otp,
            tc.tile_pool(name="osb", bufs=3) as osbp,
            tc.tile_pool(name="fin", bufs=4) as fin,
            tc.tile_pool(name="psum", bufs=2, space="PSUM") as psum,
            tc.tile_pool(name="psum3", bufs=3, space="PSUM") as psum3,
            tc.tile_pool(name="dram", bufs=1, space="DRAM") as dram,
        ):
            # ---- persistent SBUF state ----
            w_sb = const.tile([128, KC * WCOLS], BF16)
            nc.sync.dma_start(
                w_sb[:].rearrange("p (kc j) -> p kc j", kc=KC),
                wt.rearrange("(kc p) j -> p kc j", p=128),
            )
            wo_sb = const.tile([128, KC * D], BF16)
            b_row = const.tile([1, D], FP32)
            bias_sb = const.tile([128, D], FP32)

            def load_wo():
                # deferred until after batch 0's x-tile DMAs: wo/bias are not
                # read until the first out-proj (~100us in)
                nc.sync.dma_start(
                    wo_sb[:].rearrange("p (kc n) -> p kc n", kc=KC),
                    wo.rearrange("(kc p) n -> p kc n", p=128),
                )
                nc.sync.dma_start(b_row[:], bias[:])
                nc.gpsimd.partition_broadcast(bias_sb[:], b_row[:])

            # tail fast-finish constants: all-ones lhsT and a zero-padded
            # reciprocal row for the PE row-broadcast (out = sum_c rz[c,:])
            ones64 = const.tile([64, 64], BF16)
            nc.vector.memset(ones64[:], 1.0)
            rz = const.tile([64, 512], BF16)
            nc.vector.memset(rz[:], 0.0)

            q_sb = const.tile([128, T], BF16)  # [2 heads x 64, tokens] scaled
            k_sb = const.tile([128, T], BF16)
            # V token-major: [128 tok-in-chunk, (global chunk, head) x 65]
            v_sb = const.tile([128, (T // 128) * HL * 65], BF16)
            v3 = v_sb[:].rearrange("p (blk e) -> p blk e", e=65)
            nc.vector.memset(v3[:, :, 64:65], 1.0)

            a2a_in = {}
            a2a_out = {}
            for b in range(B - 1):
                a2a_in[b] = dram.tile(
                    [N_CORES, HL * HD, TPB], BF16, name=f"a2a_in{b}"
                )
                a2a_out[b] = dram.tile(
                    [N_CORES, HL * HD, TPB], BF16, name=f"a2a_out{b}"
                )
            # last batch: four quarter pieces (one per q-tile, 64 tok/core)
            # so its collectives and out-proj overlap the tail of attention
            # instead of serializing after it
            a2a_in3 = {}
            a2a_out3 = {}
            for hf in range(4):
                a2a_in3[hf] = dram.tile(
                    [N_CORES, HL * HD, 64], BF16, name=f"a2a_in3_{hf}"
                )
                a2a_out3[hf] = dram.tile(
                    [N_CORES, HL * HD, 64], BF16, name=f"a2a_out3_{hf}"
                )

            def emit_a2a(b):
                nc.gpsimd.collective_compute(
                    "AllToAll",
                    mybir.AluOpType.bypass,
                    replica_groups=[list(range(N_CORES))],
                    ins=[a2a_in[b].opt()],
                    outs=[a2a_out[b].opt()],
                )

            def emit_a2a3(hf):
                nc.gpsimd.collective_compute(
                    "AllToAll",
                    mybir.AluOpType.bypass,
                    replica_groups=[list(range(N_CORES))],
                    ins=[a2a_in3[hf].opt()],
                    outs=[a2a_out3[hf].opt()],
                )

            # groups of 2 slots: each group is one (h0, h1) pair for a kc --
            # its two S matmuls auto-pack into PE row groups 0-63/64-127, and
            # its PV is split into two C=64 half-chains (token halves of each
            # 128-chunk) that also run as concurrent row-group pairs into
            # separate single-bank accumulators (pv_a/pv_b per head)
            slot_list = [(kc, h) for kc in range(KT) for h in range(HL)]
            groups = [slot_list[g0 : g0 + 2] for g0 in range(0, len(slot_list), 2)]

            pending = []  # (b, group, p_t, pv) with S+exp emitted, PV not

            def emit_pv_flush():
                b, group, p_t, pv = pending.pop(0)
                for i, (kc, h) in enumerate(group):
                    gc = b * KT + kc
                    nc.tensor.matmul(
                        pv[h][0:65, :],
                        lhsT=v3[:, gc * HL + h, :],
                        rhs=p_t[:, i * 512 : (i + 1) * 512],
                        start=(kc == 0),
                        stop=(kc == KT - 1),
                    )

            def emit_group(b, group, pv, qt):
                # S matmuls + exp for this group; the PV matmuls are emitted
                # one group later (via pending) so the in-order PE queue never
                # head-stalls waiting on the exp of its own group
                q_off = b * NTOK + qt * TN
                width = len(group) * 512
                s_t = psum3.tile([128, 1024], FP32, tag="big3", name="s_t")
                for i, (kc, h) in enumerate(group):
                    nc.tensor.matmul(
                        s_t[:, i * 512 : (i + 1) * 512],
                        lhsT=k_sb[
                            h * 64 : (h + 1) * 64,
                            b * NTOK + kc * 128 : b * NTOK + (kc + 1) * 128,
                        ],
                        rhs=q_sb[h * 64 : (h + 1) * 64, q_off : q_off + TN],
                        start=True,
                        stop=True,
                    )
                p_t = probs.tile([128, 1024], BF16, tag="p", name="p_t")
                nc.scalar.activation(p_t[:, 0:width], s_t[:, 0:width], AF.Exp)
                pending.append((b, group, p_t, pv))
                while len(pending) > 2:
                    emit_pv_flush()

            def finish_qt(b, pv, qt):
                for h in range(HL):
                    # single copy releases the PV PSUM bank; the rest of the
                    # normalize chain runs on SBUF off the fast path
                    o_c = norm.tile([65, 512], FP32, tag="oc", name="o_c")
                    nc.vector.tensor_copy(o_c[:], pv[h][0:65, :])
                    # reciprocal on one partition is ~3.3us (512 sequential
                    # elements); DMA-reshape the 512 denominators across 128
                    # partitions so it runs in ~4 elements/lane
                    rs = norm.tile([128, 4], FP32, tag="rs", name="rs")
                    nc.sync.dma_start(rs[:], o_c[64:65, :])
                    rr = norm.tile([128, 4], FP32, tag="rr", name="rr")
                    nc.vector.reciprocal(rr[:], rs[:])
                    rec = norm.tile([1, 512], FP32, tag="rec", name="rec")
                    nc.sync.dma_start(rec[:], rr[:])
                    bc = norm.tile([64, 512], FP32, tag="bc", name="bc")
                    nc.gpsimd.partition_broadcast(bc[:], rec[:])
                    o_t = otp.tile([64, 512], BF16, tag="o", name="o_t")
                    nc.vector.tensor_mul(o_t[:], o_c[0:64, :], bc[:])
                    if b < B - 1:
                        nc.sync.dma_start(
                            a2a_in[b][
                                2 * qt : 2 * qt + 2, h * 64 : (h + 1) * 64, :
                            ].rearrange("j p e -> p j e"),
                            o_t[:].rearrange("p (j e) -> p j e", j=2),
                        )
                    else:
                        nc.sync.dma_start(
                            a2a_in3[qt][
                                :, h * 64 : (h + 1) * 64, :
                            ].rearrange("j p e -> p j e"),
                            o_t[:].rearrange("p (j e) -> p j e", j=8),
                        )

            def finish_fast(b, pv, qt):
                # tail-only finish: approx reciprocal on the denominator row
                # (no DMA reshape) + PE ones-matmul row broadcast -- ~4us
                # less serial latency than the DMA/GpSimd chain
                for h in range(HL):
                    o_c = norm.tile([65, 512], FP32, tag="oc", name="o_c")
                    nc.vector.tensor_copy(o_c[:], pv[h][0:65, :])
                    rrow = norm.tile([1, 512], FP32, tag="rrow", name="rrow")
                    nc.vector.reciprocal_approx_fast(
                        out=rrow[:], in_=o_c[64:65, :]
                    )
                    nc.vector.tensor_copy(rz[0:1, :], rrow[:])
                    bc_ps = psum3.tile([128, 1024], FP32, tag="big3", name="bc")
                    nc.tensor.matmul(
                        bc_ps[0:64, 0:512],
                        lhsT=ones64[:, :],
                        rhs=rz[:, :],
                        start=True,
                        stop=True,
                    )
                    o_t = otp.tile([64, 512], BF16, tag="o", name="o_t")
                    nc.vector.tensor_mul(o_t[:], o_c[0:64, :], bc_ps[0:64, 0:512])
                    nc.sync.dma_start(
                        a2a_in3[qt][
                            :, h * 64 : (h + 1) * 64, :
                        ].rearrange("j p e -> p j e"),
                        o_t[:].rearrange("p (j e) -> p j e", j=8),
                    )

            # ---- filler units (emitted between attention groups) ----
            # Each unit is <= ~2.2us of PE work so it fits inside the ~3us
            # exp runway the two-group s_t pipeline provides; bigger units
            # head-block the in-order PE queue and stall the exp stream.
            xts_store = {}

            def emit_xload(t):
                # prefetch: 8 x-tile DMAs, no engine work
                xts = []
                for kc in range(KC):
                    x_t = xin.tile([128, TN], BF16, tag="xt", name="x_t")
                    nc.sync.dma_start(
                        x_t[:],
                        xt[kc * 128 : (kc + 1) * 128, t * TN : (t + 1) * TN],
                    )
                    xts.append(x_t)
                xts_store[t] = xts

            def emit_qm_sub(t, m, j):
                # one complete 8-chunk projection chain over a 128-token
                # column slice: m=0 -> Q^T (scaled), m=1 -> K^T. ~0.55us of
                # PE work, small enough to ride the PE's lead over the exp
                # stream without stalling it.
                xts = xts_store[t]
                y_ps = psum3.tile([128, 1024], FP32, tag="big3", name="y_qm")
                for kc in range(KC):
                    nc.tensor.matmul(
                        y_ps[:, 0:128],
                        lhsT=w_sb[
                            :,
                            kc * WCOLS + m * 128 : kc * WCOLS + (m + 1) * 128,
                        ],
                        rhs=xts[kc][:, j * 128 : (j + 1) * 128],
                        start=kc == 0,
                        stop=kc == KC - 1,
                    )
                # epilogues on VectorE (keep ScalarE free for exp)
                c0 = t * TN + j * 128
                if m == 0:
                    nc.vector.tensor_scalar_mul(
                        q_sb[:, c0 : c0 + 128], y_ps[:, 0:128], SCALE
                    )
                else:
                    nc.vector.tensor_copy(
                        k_sb[:, c0 : c0 + 128], y_ps[:, 0:128]
                    )

            def emit_v_sub(t, s):
                # V chain for one 128-token subtile (token-major layout)
                xts = xts_store[t]
                y_ps = psum3.tile([128, 1024], FP32, tag="big3", name="y_v")
                for kc in range(KC):
                    nc.tensor.matmul(
                        y_ps[:, 0:128],
                        lhsT=xts[kc][:, s * 128 : (s + 1) * 128],
                        rhs=w_sb[:, kc * WCOLS + 256 : kc * WCOLS + WCOLS],
                        start=kc == 0,
                        stop=kc == KC - 1,
                    )
                blk = t * 4 + s
                nc.vector.tensor_copy(
                    v3[:, blk * HL : (blk + 1) * HL, 0:64],
                    y_ps[:, 0:128].rearrange("p (h d) -> p h d", h=HL),
                )

            osb_store = {}
            outt_store = {}

            def emit_op_sub(bb, m, c):
                # out-proj chain for a 128-wide output-column slice of the
                # m-th 128-token tile of batch bb
                if m == 0 and c == 0:
                    o_sb = osbp.tile(
                        [128, N_CORES * TPB], BF16, tag="osb", name="o_sb"
                    )
                    for i in range(N_CORES):
                        nc.sync.dma_start(
                            o_sb[:, i * TPB : (i + 1) * TPB], a2a_out[bb][i, :, :]
                        )
                    osb_store[bb] = o_sb
                o_sb = osb_store[bb]
                if c == 0:
                    outt_store[bb] = fin.tile(
                        [128, D], FP32, tag="outt", name="out_t"
                    )
                out_t = outt_store[bb]
                o_ps = psum3.tile([128, 1024], FP32, tag="big3", name="o_ps")
                for i in range(N_CORES):
                    nc.tensor.matmul(
                        o_ps[:, 0:128],
                        lhsT=o_sb[
                            :, i * TPB + m * 128 : i * TPB + (m + 1) * 128
                        ],
                        rhs=wo_sb[:, i * D + c * 128 : i * D + (c + 1) * 128],
                        start=(i == 0),
                        stop=(i == N_CORES - 1),
                    )
                nc.vector.tensor_add(
                    out_t[:, c * 128 : (c + 1) * 128],
                    o_ps[:, 0:128],
                    bias_sb[:, c * 128 : (c + 1) * 128],
                )
                if c == KC - 1:
                    nc.sync.dma_start(
                        out[bb * TPB + m * 128 : bb * TPB + (m + 1) * 128, :],
                        out_t[:],
                    )

            osbq_store = {}
            outq_store = {}

            def emit_op3q(p, c):
                # out-proj column-chain for quarter piece p of the last batch
                # (64 tokens, out rows 768+64p..+64)
                if c == 0:
                    o_sbq = osbp.tile([128, 512], BF16, tag="osb", name="o_sbq")
                    for i in range(N_CORES):
                        nc.sync.dma_start(
                            o_sbq[:, i * 64 : (i + 1) * 64], a2a_out3[p][i, :, :]
                        )
                    osbq_store[p] = o_sbq
                    outq_store[p] = fin.tile(
                        [128, D], FP32, tag="outt", name="outq"
                    )
                o_sbq = osbq_store[p]
                outq = outq_store[p]
                o_ps = psum3.tile([128, 1024], FP32, tag="big3", name="o_ps")
                for i in range(N_CORES):
                    nc.tensor.matmul(
                        o_ps[0:64, 0:128],
                        lhsT=o_sbq[:, i * 64 : (i + 1) * 64],
                        rhs=wo_sb[:, i * D + c * 128 : i * D + (c + 1) * 128],
                        start=(i == 0),
                        stop=(i == N_CORES - 1),
                    )
                nc.vector.tensor_add(
                    outq[0:64, c * 128 : (c + 1) * 128],
                    o_ps[0:64, 0:128],
                    bias_sb[0:64, c * 128 : (c + 1) * 128],
                )
                if c == KC - 1:
                    nc.sync.dma_start(
                        out[(B - 1) * TPB + p * 64 : (B - 1) * TPB + (p + 1) * 64, :],
                        outq[0:64, :],
                    )

            # ---- main stream ----
            fin_q = []  # (b, pv, qt) awaiting finish, carried across batches
            fillers = []  # global deque; leftovers spill into the next batch

            for b in range(B):
                if b == 0:
                    # batch 0 prologue: QKV upfront at sub-unit granularity,
                    # interleaved with qt0 attention: each group g needs only
                    # K chunk g (one k-sub) and its PV flush (2 groups later)
                    # only V chunk g (one v-sub), so after tile 0's Q the
                    # stream is [k v g k v g ...] and exp starts ~4us in
                    pv0 = [
                        psum.tile([128, 512], FP32, tag="pv", name=f"pv{h}")
                        for h in range(HL)
                    ]
                    emit_xload(0)
                    for j in range(4):
                        emit_qm_sub(0, 0, j)
                    for t in range(4):
                        if t + 1 < 4:
                            emit_xload(t + 1)
                        for j in range(4):
                            kc = 4 * t + j
                            emit_qm_sub(t, 1, j)
                            emit_v_sub(t, j)
                            emit_group(0, groups[kc], pv0, 0)
                        if t >= 1:
                            for j in range(4):
                                emit_qm_sub(t, 0, j)
                    fin_q.append((0, pv0, 0))
                    load_wo()
                    qts = [1, 2, 3]
                else:
                    qts = [0, 1, 2, 3]
                    # drain leftover units from the previous batch NOW: this
                    # batch's qt0 groups depend on its QKV fillers, and the
                    # in-order PE queue would deadlock if an S matmul queued
                    # ahead of the K/V writes it waits on
                    while fillers:
                        fillers.pop(0)()

                # QKV sub-units first (they gate the next batch's attention);
                # out-proj sub-units last so their o_sb DMAs never head-block
                # the Sync queue on a still-flying A2A
                if b < B - 1:
                    ts = range(4 * (b + 1), 4 * (b + 1) + 4)
                    for j, t in enumerate(ts):
                        if j == 0:
                            fillers.append(lambda t=t: emit_xload(t))
                        for u in range(4):
                            fillers += [
                                lambda t=t, u=u: emit_qm_sub(t, 0, u),
                                lambda t=t, u=u: emit_qm_sub(t, 1, u),
                                lambda t=t, u=u: emit_v_sub(t, u),
                            ]
                        if j < 3:
                            fillers.append(lambda t=t + 1: emit_xload(t))
                if b >= 1:
                    for m in range(2):
                        for c in range(KC):
                            fillers.append(
                                lambda bb=b - 1, m=m, c=c: emit_op_sub(bb, m, c)
                            )
                if b == B - 1:
                    # quarter pieces 0-1 of this batch's own out-proj ride as
                    # late fillers (their A2As land mid-batch)
                    for p in range(2):
                        for c in range(KC):
                            fillers.append(lambda p=p, c=c: emit_op3q(p, c))

                gcount = 0
                for qt in qts:
                    pv = [
                        psum.tile([128, 512], FP32, tag="pv", name=f"pv{h}")
                        for h in range(HL)
                    ]
                    for gi, g in enumerate(groups):
                        emit_group(b, g, pv, qt)
                        gcount += 1
                        if gi == 1 and fin_q:
                            fb, fpv, fqt = fin_q.pop(0)
                            finish_qt(fb, fpv, fqt)
                            if fb < B - 1 and fqt == 3:
                                emit_a2a(fb)
                            if fb == B - 1:
                                emit_a2a3(fqt)
                        # one ~0.55us sub-unit after every group: small enough
                        # to fit the PE's lead over the exp stream, dense
                        # enough to keep the PE p-state ramped
                        if fillers and (b < B - 1 or gcount >= 32):
                            fillers.pop(0)()
                            # xload entries cost no PE time; emit the next
                            # real unit too so the PE stays fed
                            if (
                                fillers
                                and gcount % 1 == 0
                                and len(fillers) > (len(groups) * len(qts) - gcount)
                            ):
                                fillers.pop(0)()
                    fin_q.append((b, pv, qt))

            # ---- tail: fast-finish the last q-tile, fire its quarter A2A,
            # then out-proj piece 2 (its A2A is long done, overlapping piece
            # 3's flight) and finally piece 3 ----
            while pending:
                emit_pv_flush()
            fb, fpv, fqt = fin_q.pop(0)
            finish_qt(fb, fpv, fqt)  # (3, pv, 3)
            emit_a2a3(3)
            for p in (2, 3):
                for c in range(KC):
                    emit_op3q(p, c)

    nc.compile()
    return nc


_NC_CACHE = None


def _get_nc():
    global _NC_CACHE
    if _NC_CACHE is None:
        _NC_CACHE = build_nc()
    return _NC_CACHE


def make_in_maps(x, w_qkv, w_out, b_out):
    x = np.asarray(x, dtype=np.float32)
    w_qkv = np.asarray(w_qkv, dtype=np.float32)
    w_out = np.asarray(w_out, dtype=np.float32)
    b_out = np.asarray(b_out, dtype=np.float32)

    xt_np = np.ascontiguousarray(x.reshape(T, D).T).astype(ml_dtypes.bfloat16)
    wo_np = np.ascontiguousarray(w_out.T).astype(ml_dtypes.bfloat16)
    b_np = np.ascontiguousarray(b_out.reshape(1, D))

    in_maps = []
    for c in range(N_CORES):
        rows = []
        for sec in range(3):  # q, k, v sections of w_qkv
            for hh in range(HL):
                h = HL * c + hh
                rows.append(w_qkv[sec * D + h * HD : sec * D + (h + 1) * HD, :])
        wt_np = np.ascontiguousarray(np.concatenate(rows, 0).T).astype(
            ml_dtypes.bfloat16
        )  # (1024, 384)
        in_maps.append({"xt": xt_np, "wt": wt_np, "wo": wo_np, "bias": b_np})
    return in_maps


def kernel(x, w_qkv, w_out, b_out, _trace=False, _tmpdir=None):
    in_maps = make_in_maps(x, w_qkv, w_out, b_out)
    nc = _get_nc()
    res = bass_utils.run_bass_kernel_spmd(
        nc, in_maps, core_ids=list(range(N_CORES)), trace=_trace, tmpdir=_tmpdir
    )
    # core j out rows: batches 0-2: r = b*256+u -> token b*2048 + j*256 + u;
    # batch 3 (quarter-split A2A): r = 768 + p*64 + u -> 6144 + p*512 + j*64 + u
    full = np.empty((T, D), np.float32)
    for j in range(N_CORES):
        o = np.asarray(res.results[j]["out"], dtype=np.float32)
        for b in range(B - 1):
            full[b * NTOK + j * TPB : b * NTOK + (j + 1) * TPB] = o[
                b * TPB : (b + 1) * TPB
            ]
        for p in range(4):
            dst = (B - 1) * NTOK + p * 512 + j * 64
            srcr = (B - 1) * TPB + p * 64
            full[dst : dst + 64] = o[srcr : srcr + 64]
    kernel.last_result = res
    return full.reshape(B, NTOK, D)
